# revision 1
# baseline (speedup 1.0000x reference)
"""MoE block (top-1 routing, shared FFN + per-expert LoRA) on 8 TRN2 NeuronCores.

v2: data-parallel over tokens (1024/core), weights replicated.

    logits = x @ gate_W.T + gate_b ; e* = argmax(logits)        (fp32-split)
    u      = x @ A_cat.T                 [N, 32]
    u_m    = u * onehot-mask(e*)
    inter  = relu(x @ wi_W.T + u_m @ B_cat + wi_b)              (bf16 matmul)
    out    = inter @ wo_W.T + wo_b                              (bf16 matmul)

vs v1: single-f mm1 granularity with 6 rotating PSUM banks (kills act-drain
stalls), router transposes/argmax interleaved into the mm1 main stream (hides
the PE<->DVE ping-pong), LoRA stop-passes as 4-concurrent row-tiled matmuls,
fine-grained DMA priority so mains start as early as possible, activations
split scalar/vector, bf16 output.
"""

import numpy as np
import ml_dtypes
from contextlib import ExitStack

import concourse.bass as bass
import concourse.tile as tile
from concourse import bacc, mybir
from concourse.bass_utils import run_bass_kernel_spmd
from concourse.masks import make_identity

F32 = mybir.dt.float32
F32R = mybir.dt.float32r
BF16 = mybir.dt.bfloat16
U32 = mybir.dt.uint32
BF = ml_dtypes.bfloat16

B, S, D, F, E, R = 4, 2048, 1024, 4096, 8, 4
NCORES = 8
NT = B * S          # 8192 tokens total
N = NT // NCORES    # 1024 tokens per core
ER = E * R          # 32 lora rows
KD = D // 128       # 8 contraction tiles over D
KF = F // 128       # 32 contraction tiles over F
TH = 2              # token halves (512 each)
P = 128

Relu = mybir.ActivationFunctionType.Relu
Identity = mybir.ActivationFunctionType.Identity
Add = mybir.AluOpType.add
Max = mybir.AluOpType.max
IsEq = mybir.AluOpType.is_equal

TS = [slice(th * 512, (th + 1) * 512) for th in range(TH)]


def _emit(ctx: ExitStack, tc: tile.TileContext, io: dict):
    nc = tc.nc

    consts = ctx.enter_context(tc.tile_pool(name="consts", bufs=1))
    xpool = ctx.enter_context(tc.tile_pool(name="xpool", bufs=1))
    wipool = ctx.enter_context(tc.tile_pool(name="wipool", bufs=1))
    ipool = ctx.enter_context(tc.tile_pool(name="ipool", bufs=1))
    rwork = ctx.enter_context(tc.tile_pool(name="rwork", bufs=1))
    wop = ctx.enter_context(tc.tile_pool(name="wop", bufs=2))
    outp = ctx.enter_context(tc.tile_pool(name="outp", bufs=2))
    # 2 banks: warm-up + router accumulators + mask transposes
    ppc = ctx.enter_context(tc.tile_pool(name="ppc", bufs=2, space="PSUM"))
    # 6 banks rotating for mm1/mm2 accumulation (3 f-singles in flight)
    pp = ctx.enter_context(tc.tile_pool(name="pp", bufs=6, space="PSUM"))

    # ---------- warm-up (memset on idle gpsimd; PE spins from boot) ----------
    # warm_ps lives in the pp pool so interleaved warms never touch the
    # open router accumulation banks in ppc
    warm_src = consts.tile([P, 512], BF16, tag="warm")
    nc.gpsimd.memset(warm_src, 1.0)
    warm_ps = pp.tile([P, 512], F32, tag="pb", name="warm_ps")

    def warm(n):
        for _ in range(n):
            nc.tensor.matmul(warm_ps, lhsT=warm_src[:, 0:P], rhs=warm_src,
                             start=True, stop=True)

    identity = consts.tile([P, P], F32, tag="identity")
    make_identity(nc, identity)
    identity16 = consts.tile([P, P], BF16, tag="identity16")
    nc.vector.tensor_copy(identity16, identity)

    # ---------- consts on the scalar DMA queue (lands early) ----------
    # biases [128, 64] f32: cols 0:32 wi_b by f-tile, 32:40 wo_b by d-tile,
    # 40:41 gate_b by expert partition (rows 0:8), rest zero.
    biases_sb = consts.tile([P, 64], F32, tag="biases")
    nc.scalar.dma_start(out=biases_sb, in_=io["biases"])
    wib_sb = biases_sb[:, 0:KF]
    wob_sb = biases_sb[:, KF:KF + KD]
    gb_col = biases_sb[0:E, 40:41]
    # cg [D, 72] bf16: cols 0:32 a16 (er-major), 32:40 g16, 40:64 zero,
    # 64:72 dg16  (router+lora stationaries; logits fp32-split, u single-term)
    CW = 72
    cg_big = consts.tile([P, KD * CW], BF16, tag="cg")
    nc.scalar.dma_start(out=cg_big.rearrange("p (k c) -> p k c", k=KD),
                        in_=io["cgT"].rearrange("(k p) c -> p k c", p=P))
    cg_sb = [cg_big[:, k * CW:(k + 1) * CW] for k in range(KD)]
    # bcat stacked 4x vertically (er-major rows) for row-tiled stop quads;
    # DMA issued later (after x16/dx16 on the scalar queue)
    bT4_sb = consts.tile([P, F], BF16, tag="bT4")

    # ---------- input DMAs on the sync queue, priority order ----------
    x16 = xpool.tile([P, KD * N], BF16, tag="x16")
    x16v = x16.rearrange("p (k t) -> p k t", k=KD)
    x16_src = io["xT16"].rearrange("(h k p) t -> h p k t", h=2, p=P)
    dx16 = xpool.tile([P, KD * N], BF16, tag="dx16")
    dx16v = dx16.rearrange("p (k t) -> p k t", k=KD)
    dx16_src = io["dxT16"].rearrange("(h k p) t -> h p k t", h=2, p=P)
    wi_src = io["wiT"].rearrange("(k p) f -> p k f", p=P)
    NC_WI = 8           # wi chunks: 4 f-tiles (512 f-cols) x all k each
    wi_c = [wipool.tile([P, KD * 512], BF16, tag=f"wic{c}", name=f"wic{c}")
            for c in range(NC_WI)]

    def wi_dma(c):
        nc.sync.dma_start(
            out=wi_c[c].rearrange("p (k f) -> p k f", k=KD),
            in_=wi_src[:, :, c * 512:(c + 1) * 512])

    def wi_lhsT(k, f):
        c, fr = divmod(f, 4)
        return wi_c[c][:, k * 512 + fr * P:k * 512 + (fr + 1) * P]

    # x16 halves land in parallel on the two queues (quarter-split on sync
    # so phase1 k0..1 can start earliest); dx16 + bT4 ride the scalar queue
    x16_src4 = io["xT16"].rearrange("(q k p) t -> q p k t", q=4, p=P)
    nc.sync.dma_start(out=x16v[:, 0:2], in_=x16_src4[0])
    nc.sync.dma_start(out=x16v[:, 2:4], in_=x16_src4[1])
    nc.scalar.dma_start(out=x16v[:, 4:8], in_=x16_src[1])
    for h in range(2):
        nc.scalar.dma_start(out=dx16v[:, 4 * h:4 * h + 4], in_=dx16_src[h])
    nc.scalar.dma_start(out=bT4_sb, in_=io["bT4"])
    for c in range(NC_WI - 1):
        wi_dma(c)
    nc.scalar.dma_start(
        out=wi_c[NC_WI - 1].rearrange("p (k f) -> p k f", k=KD),
        in_=wi_src[:, :, (NC_WI - 1) * 512:NC_WI * 512])

    xk = [x16[:, k * N:(k + 1) * N] for k in range(KD)]
    dxk = [dx16[:, k * N:(k + 1) * N] for k in range(KD)]

    # ---------- resident intermediates ----------
    inter_sb = [ipool.tile([P, N], BF16, tag=f"inter{f}", name=f"inter{f}")
                for f in range(KF)]
    # um_stack rows: 0:32 th0, 32:64 th1, 64:96 th0, 96:128 th1
    um_stack = rwork.tile([P, 512], BF16, tag="um")
    # router scratch (legal partition bases only: 0/32/64/96)
    ub2 = rwork.tile([64, 512], F32, tag="ub2")       # [0:32] th0, [32:64] th1
    cdlg = rwork.tile([40, 512], F32, tag="cdlg")     # [0:8] th0, [32:40] th1
    lgb = [rwork.tile([E, 512], F32, tag=f"lgb{th}", name=f"lgb{th}")
           for th in range(TH)]                       # logits + gate_b, base 0
    m32 = rwork.tile([64, 512], BF16, tag="m32")      # [0:32] th0, [32:64] th1

    # ---------- PE emission ----------
    warm(16)

    # router group: pcu[th] rows = [u | logits | pad | du | dlogits]
    pcu = [ppc.tile([CW, 512], F32, tag="pc", name=f"pcu{th}")
           for th in range(TH)]

    def router_phase1(ks):
        for k in ks:
            for th in range(TH):
                nc.tensor.matmul(pcu[th], lhsT=cg_sb[k], rhs=xk[k][:, TS[th]],
                                 start=(k == 0), stop=False)

    def router_phase2(th):
        for k in range(KD):
            nc.tensor.matmul(pcu[th], lhsT=cg_sb[k],
                             rhs=dxk[k][:, TS[th]],
                             start=False, stop=(k == KD - 1))

    def f_mains(f, ps, ks):
        for k in ks:
            for th in range(TH):
                nc.tensor.matmul(ps[th], lhsT=wi_lhsT(k, f),
                                 rhs=xk[k][:, TS[th]],
                                 start=(k == 0), stop=False)

    def quad_stops(f0, f1, ps4):
        # 4 concurrent row-tiled stop matmuls: (f0,th0)(f0,th1)(f1,th0)(f1,th1)
        for j, f in ((0, f0), (1, f0), (2, f1), (3, f1)):
            nc.tensor.matmul(ps4[j], lhsT=bT4_sb[32 * j:32 * (j + 1),
                                                 f * P:(f + 1) * P],
                             rhs=um_stack[32 * j:32 * (j + 1), :],
                             start=False, stop=True,
                             tile_position=(32 * j, 0))

    def f_acts(f, ps):
        # th0 on scalar, th1 on vector
        nc.scalar.activation(inter_sb[f][:, TS[0]], ps[0], Relu,
                             bias=wib_sb[:, f:f + 1])
        nc.vector.tensor_scalar(inter_sb[f][:, TS[1]], ps[1],
                                wib_sb[:, f:f + 1], 0.0, Add, Max)

    def f_tiles(f):
        return [pp.tile([P, 512], F32, tag="pb", name=f"p{f}_{th}")
                for th in range(TH)]

    # mask-path emitters (chunk tt = th*4 + q over 128-token chunks).
    # ptr transposes for a th land side-by-side in ONE psum bank.
    ptr_ps = {}
    pm_ps = {}
    mrep = {}

    def ptr_bank(th):
        ptr_ps[th] = ppc.tile([P, 4 * E], F32, tag="pc", name=f"ptrb{th}")

    def ptr_emit(tt):
        th, q = divmod(tt, 4)
        nc.tensor.matmul(ptr_ps[th][:, q * E:(q + 1) * E],
                         lhsT=lgb[th][:, q * P:(q + 1) * P],
                         rhs=identity[0:E, 0:E],
                         is_transpose=True, start=True, stop=True)

    mrep = {th: rwork.tile([P, 4 * ER], BF16, tag=f"mr{th}", name=f"mr{th}")
            for th in range(TH)}

    def argmax_emit(tt):
        # mask directly from (logit == rowmax): no index math needed
        th, q = divmod(tt, 4)
        chunk = ptr_ps[th][:, q * E:(q + 1) * E]
        max8 = rwork.tile([P, E], F32, tag="mx8", bufs=2, name=f"mx8_{tt}")
        nc.vector.max(out=max8, in_=chunk)
        nc.vector.tensor_scalar(
            mrep[th][:, q * ER:(q + 1) * ER].rearrange("p (e r) -> p e r", e=E),
            chunk[:, :, None].broadcast_to([P, E, R]),
            max8[:, 0:1], None, IsEq)

    def pm_emit(th):
        # one [128,128] transpose flips all 4 chunks of a th at once;
        # chunk q's mask rows land at partition base 32q (all legal)
        pm_ps[th] = ppc.tile([P, P], BF16, tag="pc", name=f"pmb{th}")
        nc.tensor.matmul(pm_ps[th], lhsT=mrep[th], rhs=identity16,
                         is_transpose=True, start=True, stop=True)

    def m32c_emit(th):
        for q in range(4):
            nc.scalar.activation(
                m32[32 * th:32 * (th + 1), q * P:(q + 1) * P],
                pm_ps[th][32 * q:32 * (q + 1), :], Identity)

    # --- phase 1 (x16 halves land in parallel on the two queues);
    # warms keep HAM at full clock across the remaining DMA latency
    router_phase1(range(2))
    warm(6)
    router_phase1(range(2, 4))
    warm(6)
    router_phase1(range(4, KD))
    warm(6)

    # --- f0+f1 mains while dx16 lands (warms bridge the wi_c0 DMA)
    warm(8)
    ps_f = {0: f_tiles(0), 1: f_tiles(1)}
    f_mains(0, ps_f[0], range(KD))
    f_mains(1, ps_f[1], range(KD))

    # --- extraction per th: scalar stages dlogits (+gate_b, free in the
    # activation bias) and u out of PSUM; DVE forms exact fp32 logits.
    # ALL pcu[th] readers are emitted before any ppc-bank reuse.
    def extract_emit(th):
        dlg_t = cdlg[0:8, :] if th == 0 else cdlg[32:40, :]
        nc.scalar.activation(dlg_t, pcu[th][64:72, :], Identity, bias=gb_col)
        nc.scalar.activation(ub2[32 * th:32 * (th + 1), :],
                             pcu[th][0:32, :], Identity)
        nc.vector.tensor_add(lgb[th], pcu[th][32:40, :], dlg_t)

    def um_emit(th):
        nc.vector.tensor_mul(um_stack[32 * th:32 * (th + 1), :],
                             ub2[32 * th:32 * (th + 1), :],
                             m32[32 * th:32 * (th + 1), :])
        nc.vector.tensor_copy(um_stack[64 + 32 * th:96 + 32 * th, :],
                              um_stack[32 * th:32 * (th + 1), :])

    # --- phase 2 th-outer: th0's bank closes first so its extraction and
    # logit transposes overlap phase2-th1
    router_phase2(0)
    extract_emit(0)
    ptr_bank(0)
    for q in range(4):
        nc.tensor.matmul(pcu[1], lhsT=cg_sb[2 * q],
                         rhs=dxk[2 * q][:, TS[1]], start=False, stop=False)
        nc.tensor.matmul(pcu[1], lhsT=cg_sb[2 * q + 1],
                         rhs=dxk[2 * q + 1][:, TS[1]],
                         start=False, stop=(q == 3))
        ptr_emit(q)
        argmax_emit(q)
    extract_emit(1)

    # --- f2 mains with th1 logit transposes + the two mask transposes
    ptr_bank(1)
    ps_f[2] = f_tiles(2)
    for k in range(KD):
        f_mains(2, ps_f[2], [k])
        if k < 2:
            ptr_emit(4 + 2 * k)
            argmax_emit(4 + 2 * k)
            ptr_emit(5 + 2 * k)
            argmax_emit(5 + 2 * k)
        elif k == 2:
            pm_emit(0)
        elif k == 3:
            m32c_emit(0)
            um_emit(0)
        elif k == 4:
            pm_emit(1)
        elif k == 5:
            m32c_emit(1)
            um_emit(1)

    # --- close f0..f2, then steady-state f3..f31
    quad_stops(0, 1, [ps_f[0][0], ps_f[0][1], ps_f[1][0], ps_f[1][1]])
    f_acts(0, ps_f[0])
    f_acts(1, ps_f[1])
    prev = 2
    for f in range(3, KF):
        ps_f[f] = f_tiles(f)
        f_mains(f, ps_f[f], range(KD))
        if f % 2 == 1:
            quad_stops(prev, f, [ps_f[prev][0], ps_f[prev][1],
                                 ps_f[f][0], ps_f[f][1]])
            f_acts(prev, ps_f[prev])
            f_acts(f, ps_f[f])
            del ps_f[prev], ps_f[f]
            prev = None
        else:
            prev = f

    # ---------- matmul 2: outT = wo @ inter + wo_b ----------
    for d in range(KD):
        wo_big = wop.tile([P, F], BF16, tag="wo", name=f"wo{d}")
        nc.sync.dma_start(out=wo_big, in_=io["woTt"][d])
        ps = [pp.tile([P, 512], F32, tag="pb", name=f"p2_{d}_{th}")
              for th in range(TH)]
        orow = io["outT"][d * P:(d + 1) * P, :]
        osb0 = outp.tile([P, 512], BF16, tag="osb0")
        osb1 = outp.tile([P, 512], BF16, tag="osb1")
        last = d == KD - 1
        ths = ([(0,), (1,)] if last else [(0, 1)])
        for grp in ths:
            for kf in range(KF):
                for th in grp:
                    nc.tensor.matmul(ps[th],
                                     lhsT=wo_big[:, kf * P:(kf + 1) * P],
                                     rhs=inter_sb[kf][:, TS[th]],
                                     start=(kf == 0), stop=(kf == KF - 1))
            if last and grp == (0,):
                # th0's act+DMA drain under th1's matmul stream
                nc.scalar.activation(osb0, ps[0], Identity,
                                     bias=wob_sb[:, d:d + 1])
                nc.gpsimd.dma_start(out=orow[:, TS[0]], in_=osb0)
        nc.vector.tensor_scalar(osb1, ps[1], wob_sb[:, d:d + 1], None, Add)
        nc.sync.dma_start(out=orow[:, TS[1]], in_=osb1)
        if not last:
            nc.scalar.activation(osb0, ps[0], Identity, bias=wob_sb[:, d:d + 1])
            nc.gpsimd.dma_start(out=orow[:, TS[0]], in_=osb0)


_CACHED_NC = None


def build_nc():
    global _CACHED_NC
    if _CACHED_NC is not None:
        return _CACHED_NC
    nc = bacc.Bacc("TRN2", target_bir_lowering=False, debug=False,
                   enable_asserts=False, num_devices=NCORES)
    decls = [
        ("xT16", [D, N], BF16, False),
        ("dxT16", [D, N], BF16, False),
        ("cgT", [D, 72], BF16, False),
        ("biases", [P, 64], F32, False),
        ("bT4", [P, F], BF16, False),
        ("wiT", [D, F], BF16, False),
        ("woTt", [KD, P, F], BF16, False),
        ("outT", [D, N], BF16, True),
    ]
    io = {}
    for name, shape, dt_, is_out in decls:
        io[name] = nc.dram_tensor(
            name, shape, dt_, kind="ExternalOutput" if is_out else "ExternalInput"
        ).ap()
    with tile.TileContext(nc) as tc:
        with ExitStack() as ctx:
            _emit(ctx, tc, io)
    nc.compile()
    _CACHED_NC = nc
    return nc


def make_in_maps(inputs: dict) -> list[dict]:
    f32 = np.float32
    x = np.ascontiguousarray(np.asarray(inputs["hidden_states"], f32).reshape(NT, D))
    gT = np.asarray(inputs["gate_W"], f32).T                                # [D, E]
    aT = np.asarray(inputs["lora_A"], f32).reshape(ER, D).T                 # [D, 32]
    ga = np.concatenate([aT, gT], axis=1)                                   # [D, 40]
    ga16 = ga.astype(BF)
    dga16 = (ga - ga16.astype(f32)).astype(BF)
    cgT = np.ascontiguousarray(np.concatenate(
        [ga16, np.zeros((D, 24), BF), dga16[:, 32:40]], axis=1))            # [D, 72]
    biases = np.zeros((P, 64), f32)
    biases[:, 0:KF] = np.asarray(inputs["wi_b"], f32).reshape(KF, P).T
    biases[:, KF:KF + KD] = np.asarray(inputs["wo_b"], f32).reshape(KD, P).T
    biases[0:E, 40] = np.asarray(inputs["gate_b"], f32)
    bT = np.asarray(inputs["lora_B"], f32).transpose(0, 2, 1).reshape(ER, F)
    bT4 = np.ascontiguousarray(np.tile(bT.astype(BF), (4, 1)))              # [128, F]
    wiT = np.ascontiguousarray(np.asarray(inputs["wi_W"], f32).T.astype(BF))  # [D, F]
    woT = np.asarray(inputs["wo_W"], f32).T.astype(BF)                      # [F, D]
    # pre-tiled to SBUF layout: woTt[d, p, kf*128+j] = woT[kf*128+p, d*128+j]
    woTt = np.ascontiguousarray(
        woT.reshape(KF, P, KD, P).transpose(2, 1, 0, 3).reshape(KD, P, F))

    in_maps = []
    for c in range(NCORES):
        xT32 = np.ascontiguousarray(x[c * N:(c + 1) * N].T)                 # [D, N]
        xT16 = xT32.astype(BF)
        dxT16 = (xT32 - xT16.astype(f32)).astype(BF)
        in_maps.append({
            "xT16": np.ascontiguousarray(xT16),
            "dxT16": np.ascontiguousarray(dxT16),
            "cgT": cgT, "biases": biases, "bT4": bT4,
            "wiT": wiT, "woTt": woTt,
        })
    return in_maps


def kernel(**inputs) -> np.ndarray:
    nc = build_nc()
    in_maps = make_in_maps(inputs)
    res = run_bass_kernel_spmd(nc, in_maps, core_ids=list(range(NCORES)))
    out = np.empty((NT, D), np.float32)
    for c in range(NCORES):
        out[c * N:(c + 1) * N] = res.results[c]["outT"].T.astype(np.float32)
    return out.reshape(B, S, D)

